# revision 45
# baseline (speedup 1.0000x reference)
"""MoE layer (nn_MoELayer_6923487282556) on 8 Trainium2 cores.

Strategy (expert-parallel, per sharding hint):
  Launch 1 (data-parallel router): each core takes N/8 = 2048 tokens
    and computes raw router logits in one fp16 matmul pass (fp32 PSUM).
    The launch is DMA-bound (8.4MB x stream; big-descriptor DMAs reach
    ~390 GB/s), so x ships fp16-only.
  Host dispatch ("all-to-all"): softmax/top-2/combine weights from the
    device logits (recomputing exactly the ~800 tokens whose top2/top3
    logit gap is within reach of the ~1e-4 fp16 logit noise -- the
    1.4e-2 threshold is ~100 sigma -- so the routed SET matches exact
    fp32 deterministically),
    then gather tokens per expert, padded to the max expert load.
  Launch 2 (expert-parallel FFN): core e owns expert e. Computes
    silu(x@gwT) * (x@uwT) @ dwT scaled by the combine weight, in fp16
    with fp32 PSUM accumulation (1 PE cycle/row + fast-weight-load).
    PE-bound at ~98% tensor-engine occupancy; ~5.3e-4 relative error
    end-to-end (fp16 FFN).
  Host: scatter-add the two expert contributions per token.

Everything is transposed ([feature, token] layout) so no on-device
transposes are needed anywhere. All DRAM operands are laid out on the
host so each SBUF partition reads long contiguous bursts. Small output
DMAs ride the ACT engine's HWDGE ring so they never queue behind the
bulk streams on the sync ring.
"""

import numpy as np

import concourse.bass as bass
import concourse.tile as tile
from concourse import bacc, mybir
from concourse.bass_utils import run_bass_kernel_spmd

F32 = mybir.dt.float32
F32R = mybir.dt.float32r
AF = mybir.ActivationFunctionType
ALU = mybir.AluOpType
AX = mybir.AxisListType

N_CORES = 8
B, L, D = 4, 4096, 2048
N = B * L            # 16384 tokens
E = 8                # experts (== cores)
H = 3072             # ffn hidden
TOK_SHARD = N // N_CORES
KC = D // 128        # 16 contraction chunks over D
MC = H // 128        # 24 chunks over H
DC = D // 128        # 16 output chunks over D

# When set (by test.py) each launch's execution is wrapped with the axon
# NTFF profile hook and traces land in PROFILE_DIR/launch{1,2}.
PROFILE_DIR = None

_cache = {}


def _run(nc, in_maps, tag):
    core_ids = list(range(N_CORES))
    if PROFILE_DIR is None:
        return run_bass_kernel_spmd(nc, in_maps, core_ids).results
    import os
    import tempfile
    from trn_agent_boot.trn_boot import _ntff_profile_via_ctypes

    hook = _ntff_profile_via_ctypes("/opt/axon/libaxon_pjrt.so")
    # warm-up (NEFF compile) inside a throwaway profile capture so any
    # profile artifacts it produces can never pollute the real out_dir
    # (a stray warmup NTFF there would double-count the launch).
    trash = tempfile.mkdtemp(prefix="ntff_warmup_")
    with hook(trash, [0]):
        run_bass_kernel_spmd(nc, in_maps, core_ids)
    out_dir = os.path.join(PROFILE_DIR, tag)
    os.makedirs(out_dir, exist_ok=True)
    with hook(out_dir, [0]):
        res = run_bass_kernel_spmd(nc, in_maps, core_ids).results
    return res


def _build_router():
    """Per core: xs_hi [128, KC, 2048] fp16 of the token shard
    (partition-blocked x.T), rw [D, 8] fp16.  Computes raw logits =
    xh @ rw in one fp16 pass (fp32 PSUM) and ships them to the host,
    which does softmax/top-2/combine (dispatch logic) and exactly
    recomputes the few tokens whose top2/top3 logit gap is within
    reach of the ~1e-4 fp16 logit noise.  This launch is DMA-bound on
    the 8.4MB x stream.  Output logit [2048, 8] fp32."""
    F16 = mybir.dt.float16
    XT = 256  # tokens per x tile (A/B'd vs 512: 256 measures ~3us faster)
    NT = TOK_SHARD // XT
    nc = bacc.Bacc("TRN2", target_bir_lowering=False, debug=False,
                   num_devices=N_CORES)
    # one un-sliced DMA per tile: per-partition contiguous run = KC*XT*2
    # = 8KB, vs 1KB when slicing by k (descriptor overhead capped the
    # stream at ~170 GB/s instead of ~218)
    xs_hi = nc.dram_tensor("xs_hi", [NT, 128, KC, XT], F16,
                           kind="ExternalInput").ap()
    rw = nc.dram_tensor("rw", [D, E], F16, kind="ExternalInput").ap()
    logit = nc.dram_tensor("logit", [TOK_SHARD, E], F32,
                           kind="ExternalOutput").ap()

    with tile.TileContext(nc) as tc:
        with (
            # bufs = all 8 tiles resident: every x DMA issues up front so
            # the stream never stalls on buffer reuse (SBUF is nearly
            # empty in this launch; 8 x 1MB tiles is fine)
            tc.tile_pool(name="xp", bufs=8) as xp,
            tc.tile_pool(name="wp", bufs=1) as wp,
            tc.tile_pool(name="pp", bufs=4, space="PSUM") as pp,
            tc.tile_pool(name="sp", bufs=8) as sp,
        ):
            # rw + logit outs ride the ACT engine's HWDGE ring so they
            # never queue behind the x stream on the sync ring
            rw_t = wp.tile([128, KC, E], F16, name="rw")
            nc.scalar.dma_start(rw_t[:], rw.rearrange("(k p) e -> p k e",
                                                   p=128))
            # issue ALL x-tile DMAs up front, alternating between the two
            # HWDGE rings: one ring only keeps ~3 jobs in flight, which
            # starves the DMA engines at job boundaries; two rings keep
            # the 16 shared engines continuously fed
            tiles_x = []
            with tc.high_priority():
                for ti in range(NT):
                    xh = xp.tile([128, KC, XT], F16, tag="xh", name="xh")
                    if ti == NT - 1:
                        # last tile via the GPSIMD software-DGE path: a
                        # third DMA ring, so each HWDGE ring stays within
                        # its ~3-job in-flight depth and streams gapless;
                        # this tile is consumed last so SWDGE's lower
                        # throughput is off the critical path
                        eng = nc.gpsimd
                    else:
                        eng = nc.sync if ti % 2 == 0 else nc.scalar
                    eng.dma_start(xh[:], xs_hi[ti])
                    tiles_x.append(xh)
            for ti in range(NT):
                xh = tiles_x[ti]
                for sub in range(XT // 128):
                    ss = slice(sub * 128, (sub + 1) * 128)
                    ps = pp.tile([128, E], F32, tag="ps")
                    for k in range(KC):
                        nc.tensor.matmul(ps[:], xh[:, k, ss], rw_t[:, k, :],
                                         start=(k == 0), stop=(k == KC - 1))
                    l = sp.tile([128, E], F32, tag="l")
                    nc.vector.tensor_copy(l[:], ps[:])
                    t0 = ti * XT + sub * 128
                    nc.scalar.dma_start(logit[t0:t0 + 128, :], l[:])
    nc.compile()
    return nc


T_MAX = 1536


def _tile_sizes(cap):
    """First tile is small (512) so the launch's DMA fill (x chunk +
    first gate/up weight tiles) is ~3.4MB instead of ~7.3MB -- the PE
    starts ~20us sooner.  Tiles must stay >=~450 tokens: each tile
    streams the full 37.75MB weight set, and below ~450 tokens the
    weight DMA (~174us) outruns the tile's compute.  Max 1536 tokens
    (SBUF: h tile + x tile + weight buffers).  NOTE: other tilings that
    look better on paper ([1024,1024,1024,1183], bigger dw prefetch,
    split first x DMA) all perturbed the tile scheduler into schedules
    200-400us slower -- this configuration is measured-good."""
    if cap <= T_MAX:
        return [cap]
    tiles = [512]
    rem = cap - 512
    while rem > T_MAX + 450:
        tiles.append(T_MAX)
        rem -= T_MAX
    if rem > T_MAX:
        tiles.append(rem - rem // 2)
        tiles.append(rem // 2)
    else:
        tiles.append(rem)
    return tiles


def _chunks(T):
    # matmul free dim caps at 512 (one PSUM bank of fp32 output per MM;
    # wider outputs fail walrus codegen)
    chunks = []
    rem = T
    while rem:
        c = min(512, rem)
        chunks.append(c)
        rem -= c
    # a trailing narrow chunk (<256 cols) can't hide the ~53ns LDWEIGHTS
    # behind its matmuls (1152 MMs/chunk -> ~10us tax); rebalance the
    # last two chunks so both stay wide enough
    if len(chunks) >= 2 and chunks[-1] < 256:
        tot = chunks[-1] + chunks[-2]
        chunks[-2] = tot - tot // 2
        chunks[-1] = tot // 2
    return chunks


def _build_ffn(cap, tiles):
    """Per core (expert e):
      xg   [128, KC*cap] fp16  gathered x.T, packed tile-major: tile t's
           block is [128, KC, T_t] flattened, so each tile is ONE DMA
           with a KC*T_t*2-byte (8-48KB) contiguous run per partition
           (big descriptors stream at ~390 GB/s vs ~170 for 1-3KB runs)
      gw/uw [MC, 128, KC, 128] fp16  partition-blocked transposed weights
      dw   [DC, 128, MC, 128] fp16
      wrep [128, cap] f32  combine weights replicated over partitions
    Output outT [D, cap] f32 = (combine_w * expert_out).T."""
    F16 = mybir.dt.float16
    nc = bacc.Bacc("TRN2", target_bir_lowering=False, debug=False,
                   num_devices=N_CORES)
    xg = nc.dram_tensor("xg", [128, KC * cap], F16,
                        kind="ExternalInput").ap()
    gw = nc.dram_tensor("gw", [MC, 128, KC, 128], F16,
                        kind="ExternalInput").ap()
    uw = nc.dram_tensor("uw", [MC, 128, KC, 128], F16,
                        kind="ExternalInput").ap()
    dw = nc.dram_tensor("dw", [DC, 128, MC, 128], F16,
                        kind="ExternalInput").ap()
    wrep = nc.dram_tensor("wrep", [128, cap], F32, kind="ExternalInput").ap()
    outT = nc.dram_tensor("outT", [D, cap], F32, kind="ExternalOutput").ap()

    with tile.TileContext(nc) as tc:
        with (
            tc.tile_pool(name="xp", bufs=1) as xp,
            tc.tile_pool(name="gp", bufs=3) as gp,
            tc.tile_pool(name="up", bufs=3) as up,
            tc.tile_pool(name="dp", bufs=4) as dp,
            tc.tile_pool(name="hp", bufs=1) as hp,
            tc.tile_pool(name="sg", bufs=2) as sgp,
            tc.tile_pool(name="op", bufs=6) as opl,
            tc.tile_pool(name="wpl", bufs=2) as wpl,
            tc.tile_pool(name="ps", bufs=2, space="PSUM") as ps,
        ):
            def _xt_dma(t0, T):
                # one DMA per x tile: per-partition contiguous KC*T*2
                # bytes on both sides -> near-peak DMA rate.  Rides the
                # ACT ring: on the sync ring this 4.7MB job head-of-line
                # blocks the down-phase dw stream (~5us stall per tile
                # boundary); on the ACT ring it only delays final output
                # writes, which nothing waits on.
                xt = xp.tile([128, KC, T], F16, tag="xt", name="xt")
                nc.scalar.dma_start(
                    xt[:], xg[:, KC * t0:KC * (t0 + T)].rearrange(
                        "p (k t) -> p k t", k=KC))
                return xt

            DW_PRE = 4  # == dp bufs
            t0 = 0
            xt = _xt_dma(0, tiles[0])
            for i, T in enumerate(tiles):
                chunks = _chunks(T)
                wt = wpl.tile([128, T], F32, tag="wt")
                nc.scalar.dma_start(wt[:], wrep[:, t0:t0 + T])
                # one h tile per <=512-token chunk: keeps every tile under
                # 64KB/partition (a single [128, MC, 1536] fp16 tile faults
                # the exec unit)
                h_tiles = [hp.tile([128, MC, cl], F16, tag=f"h{ci}",
                                   name=f"h{ci}")
                           for ci, cl in enumerate(chunks)]
                for m in range(MC):
                    gw_t = gp.tile([128, KC, 128], F16, tag="gw")
                    nc.sync.dma_start(gw_t[:], gw[m])
                    uw_t = up.tile([128, KC, 128], F16, tag="uw")
                    nc.sync.dma_start(uw_t[:], uw[m])
                    c0 = 0
                    for ci, cl in enumerate(chunks):
                        gps = ps.tile([128, cl], F32, tag="gps", bufs=2)
                        for k in range(KC):
                            nc.tensor.matmul(
                                gps[:], gw_t[:, k, :], xt[:, k, c0:c0 + cl],
                                start=(k == 0), stop=(k == KC - 1))
                        ups = ps.tile([128, cl], F32, tag="ups", bufs=2)
                        for k in range(KC):
                            nc.tensor.matmul(
                                ups[:], uw_t[:, k, :], xt[:, k, c0:c0 + cl],
                                start=(k == 0), stop=(k == KC - 1))
                        sg = sgp.tile([128, cl], F16, tag="sg")
                        nc.scalar.activation(sg[:], gps[:], AF.Silu)
                        nc.vector.tensor_mul(h_tiles[ci][:, m, :],
                                             sg[:], ups[:])
                        c0 += cl
                # issue the first few dw DMAs, then the NEXT tile's x DMA,
                # before entering the d-loop: the x DMA instruction blocks
                # the sync ring until this tile's gate/up frees the xt
                # buffer, and the remaining dw issues resume right after --
                # so the next x streams during this tile's down phase
                # instead of stalling its start (~4us/tile otherwise)
                dw_pre = []
                for d in range(min(DW_PRE, DC)):
                    dw_t = dp.tile([128, MC, 128], F16, tag="dw")
                    nc.sync.dma_start(dw_t[:], dw[d])
                    dw_pre.append(dw_t)
                if i + 1 < len(tiles):
                    xt = _xt_dma(t0 + T, tiles[i + 1])
                for d in range(DC):
                    if d < len(dw_pre):
                        dw_t = dw_pre[d]
                    else:
                        dw_t = dp.tile([128, MC, 128], F16, tag="dw")
                        nc.sync.dma_start(dw_t[:], dw[d])
                    c0 = 0
                    for ci, cl in enumerate(chunks):
                        ops = ps.tile([128, cl], F32, tag="ops")
                        for m in range(MC):
                            nc.tensor.matmul(
                                ops[:], dw_t[:, m, :],
                                h_tiles[ci][:, m, :],
                                start=(m == 0), stop=(m == MC - 1))
                        ot = opl.tile([128, cl], F32, tag="ot")
                        nc.vector.tensor_mul(ot[:], ops[:], wt[:, c0:c0 + cl])
                        nc.scalar.dma_start(
                            outT[d * 128:(d + 1) * 128, t0 + c0:t0 + c0 + cl],
                            ot[:])
                        c0 += cl
                t0 += T
    nc.compile()
    return nc


def _pblock(a):
    """[R, C] with R = r*128 -> [128, r, C] so each SBUF partition reads
    contiguous data."""
    r = a.shape[0] // 128
    return np.ascontiguousarray(
        a.reshape(r, 128, a.shape[1]).transpose(1, 0, 2))


def kernel(x, router_w, gate_w, up_w, down_w):
    x = np.asarray(x, np.float32)
    router_w = np.asarray(router_w, np.float32)
    gate_w = np.asarray(gate_w, np.float32)
    up_w = np.asarray(up_w, np.float32)
    down_w = np.asarray(down_w, np.float32)

    import ml_dtypes
    BF = ml_dtypes.bfloat16

    x_flat = np.ascontiguousarray(x.reshape(N, D))
    rwT = np.ascontiguousarray(router_w.T)  # [D, E]
    rw_16 = rwT.astype(np.float16)

    # ---- launch 1: router logits (data-parallel over tokens) ----
    if "router" not in _cache:
        _cache["router"] = _build_router()
    nc_r = _cache["router"]
    xT_hi = x_flat.T.astype(np.float16)

    def _tile_major(a):  # [D, TOK_SHARD] -> [NT, 128, KC, 256]
        return np.ascontiguousarray(
            a.reshape(KC, 128, TOK_SHARD // 256, 256).transpose(2, 1, 0, 3))

    in_maps = [
        {
            "xs_hi": _tile_major(
                xT_hi[:, c * TOK_SHARD:(c + 1) * TOK_SHARD]),
            "rw": rw_16,
        }
        for c in range(N_CORES)
    ]
    res_r = _run(nc_r, in_maps, "launch1")
    logits = np.concatenate(
        [res_r[c]["logit"] for c in range(N_CORES)], 0).astype(np.float64)

    # Host dispatch math (softmax / top-2 / combine weights).  Device
    # logits carry ~1e-4 rms fp16 noise; the top-2 SET only flips when
    # the true top2/top3 logit gap is comparable, so recompute exactly
    # (in fp64, from the fp32 inputs) every token whose observed gap is
    # < 1.4e-2 (>100 sigma of the gap noise).  Deterministic, and host
    # cost is a few MFLOP (~800 tokens).
    ls = np.sort(logits, axis=1)
    risk = np.flatnonzero(ls[:, -2] - ls[:, -3] < 1.4e-2)
    if risk.size:
        logits[risk] = x_flat[risk].astype(np.float64) @ rwT.astype(np.float64)
    lmax = logits.max(axis=1, keepdims=True)
    p = np.exp(logits - lmax)
    p /= p.sum(axis=1, keepdims=True)
    top2 = np.argsort(-p, axis=1)[:, :2]
    rows = np.arange(N)
    p1 = p[rows, top2[:, 0]]
    p2 = p[rows, top2[:, 1]]
    combine = np.zeros((N, E), np.float32)
    combine[rows, top2[:, 0]] = (p1 / (p1 + p2)).astype(np.float32)
    combine[rows, top2[:, 1]] = (p2 / (p1 + p2)).astype(np.float32)

    # ---- host dispatch: token lists per expert, padded to capacity ----
    idx = [np.flatnonzero(combine[:, e] > 0.0) for e in range(E)]
    max_cnt = max(len(i) for i in idx)
    cap = max(512, max_cnt)
    tiles = _tile_sizes(cap)

    gw16 = gate_w.astype(np.float16)
    uw16 = up_w.astype(np.float16)
    dw16 = down_w.astype(np.float16)
    x16 = x_flat.astype(np.float16)
    in_maps = []
    for e in range(E):
        cnt = len(idx[e])
        xg = np.zeros((D, cap), np.float16)
        xg[:, :cnt] = x16[idx[e]].T
        # pack tile-major so each tile's x is one contiguous block per
        # partition (see _build_ffn docstring)
        xgp = _pblock(xg)  # [128, KC, cap]
        blocks, t0 = [], 0
        for T in tiles:
            blocks.append(np.ascontiguousarray(
                xgp[:, :, t0:t0 + T]).reshape(128, KC * T))
            t0 += T
        xg_flat = np.ascontiguousarray(np.concatenate(blocks, axis=1))
        wvec = np.zeros(cap, np.float32)
        wvec[:cnt] = combine[idx[e], e]
        wrep = np.ascontiguousarray(np.broadcast_to(wvec, (128, cap)))
        # gw[m, p, k, c] = gate_w[e][m*128+c, k*128+p]
        gwb = np.ascontiguousarray(
            gw16[e].reshape(MC, 128, KC, 128).transpose(0, 3, 2, 1))
        uwb = np.ascontiguousarray(
            uw16[e].reshape(MC, 128, KC, 128).transpose(0, 3, 2, 1))
        # dw[d, p, m, c] = down_w[e][d*128+c, m*128+p]
        dwb = np.ascontiguousarray(
            dw16[e].reshape(DC, 128, MC, 128).transpose(0, 3, 2, 1))
        in_maps.append({"xg": xg_flat, "gw": gwb, "uw": uwb, "dw": dwb,
                        "wrep": wrep})

    key = ("ffn", cap)
    if key not in _cache:
        _cache[key] = _build_ffn(cap, tiles)
    nc_f = _cache[key]
    res_f = _run(nc_f, in_maps, "launch2")

    # ---- host scatter-add ("all-to-all" return) ----
    out = np.zeros((N, D), np.float32)
    for e in range(E):
        cnt = len(idx[e])
        if cnt:
            out[idx[e]] += res_f[e]["outT"][:, :cnt].T
    return out.reshape(B, L, D)



# revision 46
# speedup vs baseline: 1.0026x; 1.0026x over previous
"""MoE layer (nn_MoELayer_6923487282556) on 8 Trainium2 cores.

Strategy (expert-parallel, per sharding hint):
  Launch 1 (data-parallel router): each core takes N/8 = 2048 tokens
    and computes raw router logits in one fp16 matmul pass (fp32 PSUM).
    The launch is DMA-bound (8.4MB x stream; big-descriptor DMAs reach
    ~390 GB/s), so x ships fp16-only.
  Host dispatch ("all-to-all"): softmax/top-2/combine weights from the
    device logits (recomputing exactly the ~800 tokens whose top2/top3
    logit gap is within reach of the ~1e-4 fp16 logit noise -- the
    1.4e-2 threshold is ~100 sigma -- so the routed SET matches exact
    fp32 deterministically),
    then gather tokens per expert, padded to the max expert load.
  Launch 2 (expert-parallel FFN): core e owns expert e. Computes
    silu(x@gwT) * (x@uwT) @ dwT scaled by the combine weight, in fp16
    with fp32 PSUM accumulation (1 PE cycle/row + fast-weight-load).
    PE-bound at ~98% tensor-engine occupancy; ~5.3e-4 relative error
    end-to-end (fp16 FFN).
  Host: scatter-add the two expert contributions per token.

Everything is transposed ([feature, token] layout) so no on-device
transposes are needed anywhere. All DRAM operands are laid out on the
host so each SBUF partition reads long contiguous bursts. Small output
DMAs ride the ACT engine's HWDGE ring so they never queue behind the
bulk streams on the sync ring.
"""

import numpy as np

import concourse.bass as bass
import concourse.tile as tile
from concourse import bacc, mybir
from concourse.bass_utils import run_bass_kernel_spmd

F32 = mybir.dt.float32
F32R = mybir.dt.float32r
AF = mybir.ActivationFunctionType
ALU = mybir.AluOpType
AX = mybir.AxisListType

N_CORES = 8
B, L, D = 4, 4096, 2048
N = B * L            # 16384 tokens
E = 8                # experts (== cores)
H = 3072             # ffn hidden
TOK_SHARD = N // N_CORES
KC = D // 128        # 16 contraction chunks over D
MC = H // 128        # 24 chunks over H
DC = D // 128        # 16 output chunks over D

# When set (by test.py) each launch's execution is wrapped with the axon
# NTFF profile hook and traces land in PROFILE_DIR/launch{1,2}.
PROFILE_DIR = None

_cache = {}


def _run(nc, in_maps, tag):
    core_ids = list(range(N_CORES))
    if PROFILE_DIR is None:
        return run_bass_kernel_spmd(nc, in_maps, core_ids).results
    import os
    import tempfile
    from trn_agent_boot.trn_boot import _ntff_profile_via_ctypes

    hook = _ntff_profile_via_ctypes("/opt/axon/libaxon_pjrt.so")
    # warm-up (NEFF compile) inside a throwaway profile capture so any
    # profile artifacts it produces can never pollute the real out_dir
    # (a stray warmup NTFF there would double-count the launch).
    trash = tempfile.mkdtemp(prefix="ntff_warmup_")
    with hook(trash, [0]):
        run_bass_kernel_spmd(nc, in_maps, core_ids)
    out_dir = os.path.join(PROFILE_DIR, tag)
    os.makedirs(out_dir, exist_ok=True)
    with hook(out_dir, [0]):
        res = run_bass_kernel_spmd(nc, in_maps, core_ids).results
    return res


def _build_router():
    """Per core: xs_hi [128, KC, 2048] fp16 of the token shard
    (partition-blocked x.T), rw [D, 8] fp16.  Computes raw logits =
    xh @ rw in one fp16 pass (fp32 PSUM) and ships them to the host,
    which does softmax/top-2/combine (dispatch logic) and exactly
    recomputes the few tokens whose top2/top3 logit gap is within
    reach of the ~1e-4 fp16 logit noise.  This launch is DMA-bound on
    the 8.4MB x stream.  Output logit [2048, 8] fp32."""
    F16 = mybir.dt.float16
    XT = 256  # tokens per x tile (A/B'd vs 512: 256 measures ~3us faster)
    NT = TOK_SHARD // XT
    nc = bacc.Bacc("TRN2", target_bir_lowering=False, debug=False,
                   num_devices=N_CORES)
    # one un-sliced DMA per tile: per-partition contiguous run = KC*XT*2
    # = 8KB, vs 1KB when slicing by k (descriptor overhead capped the
    # stream at ~170 GB/s instead of ~218)
    xs_hi = nc.dram_tensor("xs_hi", [NT, 128, KC, XT], F16,
                           kind="ExternalInput").ap()
    rw = nc.dram_tensor("rw", [D, E], F16, kind="ExternalInput").ap()
    logit = nc.dram_tensor("logit", [TOK_SHARD, E], F32,
                           kind="ExternalOutput").ap()

    with tile.TileContext(nc) as tc:
        with (
            # bufs = all 8 tiles resident: every x DMA issues up front so
            # the stream never stalls on buffer reuse (SBUF is nearly
            # empty in this launch; 8 x 1MB tiles is fine)
            tc.tile_pool(name="xp", bufs=8) as xp,
            tc.tile_pool(name="wp", bufs=1) as wp,
            tc.tile_pool(name="pp", bufs=4, space="PSUM") as pp,
            tc.tile_pool(name="sp", bufs=8) as sp,
        ):
            # rw + logit outs ride the ACT engine's HWDGE ring so they
            # never queue behind the x stream on the sync ring
            rw_t = wp.tile([128, KC, E], F16, name="rw")
            nc.scalar.dma_start(rw_t[:], rw.rearrange("(k p) e -> p k e",
                                                   p=128))
            # issue ALL x-tile DMAs up front, alternating between the two
            # HWDGE rings: one ring only keeps ~3 jobs in flight, which
            # starves the DMA engines at job boundaries; two rings keep
            # the 16 shared engines continuously fed
            tiles_x = []
            with tc.high_priority():
                for ti in range(NT):
                    xh = xp.tile([128, KC, XT], F16, tag="xh", name="xh")
                    eng = nc.sync if ti % 2 == 0 else nc.scalar
                    eng.dma_start(xh[:], xs_hi[ti])
                    tiles_x.append(xh)
            for ti in range(NT):
                xh = tiles_x[ti]
                for sub in range(XT // 128):
                    ss = slice(sub * 128, (sub + 1) * 128)
                    ps = pp.tile([128, E], F32, tag="ps")
                    for k in range(KC):
                        nc.tensor.matmul(ps[:], xh[:, k, ss], rw_t[:, k, :],
                                         start=(k == 0), stop=(k == KC - 1))
                    l = sp.tile([128, E], F32, tag="l")
                    nc.vector.tensor_copy(l[:], ps[:])
                    t0 = ti * XT + sub * 128
                    nc.scalar.dma_start(logit[t0:t0 + 128, :], l[:])
    nc.compile()
    return nc


T_MAX = 1536


def _tile_sizes(cap):
    """First tile is small (512) so the launch's DMA fill (x chunk +
    first gate/up weight tiles) is ~3.4MB instead of ~7.3MB -- the PE
    starts ~20us sooner.  Tiles must stay >=~450 tokens: each tile
    streams the full 37.75MB weight set, and below ~450 tokens the
    weight DMA (~174us) outruns the tile's compute.  Max 1536 tokens
    (SBUF: h tile + x tile + weight buffers).  NOTE: other tilings that
    look better on paper ([1024,1024,1024,1183], bigger dw prefetch,
    split first x DMA) all perturbed the tile scheduler into schedules
    200-400us slower -- this configuration is measured-good."""
    if cap <= T_MAX:
        return [cap]
    tiles = [512]
    rem = cap - 512
    while rem > T_MAX + 450:
        tiles.append(T_MAX)
        rem -= T_MAX
    if rem > T_MAX:
        tiles.append(rem - rem // 2)
        tiles.append(rem // 2)
    else:
        tiles.append(rem)
    return tiles


def _chunks(T):
    # matmul free dim caps at 512 (one PSUM bank of fp32 output per MM;
    # wider outputs fail walrus codegen)
    chunks = []
    rem = T
    while rem:
        c = min(512, rem)
        chunks.append(c)
        rem -= c
    # a trailing narrow chunk (<256 cols) can't hide the ~53ns LDWEIGHTS
    # behind its matmuls (1152 MMs/chunk -> ~10us tax); rebalance the
    # last two chunks so both stay wide enough
    if len(chunks) >= 2 and chunks[-1] < 256:
        tot = chunks[-1] + chunks[-2]
        chunks[-2] = tot - tot // 2
        chunks[-1] = tot // 2
    return chunks


def _build_ffn(cap, tiles):
    """Per core (expert e):
      xg   [128, KC*cap] fp16  gathered x.T, packed tile-major: tile t's
           block is [128, KC, T_t] flattened, so each tile is ONE DMA
           with a KC*T_t*2-byte (8-48KB) contiguous run per partition
           (big descriptors stream at ~390 GB/s vs ~170 for 1-3KB runs)
      gw/uw [MC, 128, KC, 128] fp16  partition-blocked transposed weights
      dw   [DC, 128, MC, 128] fp16
      wrep [128, cap] f32  combine weights replicated over partitions
    Output outT [D, cap] f32 = (combine_w * expert_out).T."""
    F16 = mybir.dt.float16
    nc = bacc.Bacc("TRN2", target_bir_lowering=False, debug=False,
                   num_devices=N_CORES)
    xg = nc.dram_tensor("xg", [128, KC * cap], F16,
                        kind="ExternalInput").ap()
    gw = nc.dram_tensor("gw", [MC, 128, KC, 128], F16,
                        kind="ExternalInput").ap()
    uw = nc.dram_tensor("uw", [MC, 128, KC, 128], F16,
                        kind="ExternalInput").ap()
    dw = nc.dram_tensor("dw", [DC, 128, MC, 128], F16,
                        kind="ExternalInput").ap()
    wrep = nc.dram_tensor("wrep", [128, cap], F32, kind="ExternalInput").ap()
    outT = nc.dram_tensor("outT", [D, cap], F32, kind="ExternalOutput").ap()

    with tile.TileContext(nc) as tc:
        with (
            tc.tile_pool(name="xp", bufs=1) as xp,
            tc.tile_pool(name="gp", bufs=3) as gp,
            tc.tile_pool(name="up", bufs=3) as up,
            tc.tile_pool(name="dp", bufs=4) as dp,
            tc.tile_pool(name="hp", bufs=1) as hp,
            tc.tile_pool(name="sg", bufs=2) as sgp,
            tc.tile_pool(name="op", bufs=6) as opl,
            tc.tile_pool(name="wpl", bufs=2) as wpl,
            tc.tile_pool(name="ps", bufs=2, space="PSUM") as ps,
        ):
            def _xt_dma(t0, T):
                # one DMA per x tile: per-partition contiguous KC*T*2
                # bytes on both sides -> near-peak DMA rate.  Rides the
                # ACT ring: on the sync ring this 4.7MB job head-of-line
                # blocks the down-phase dw stream (~5us stall per tile
                # boundary); on the ACT ring it only delays final output
                # writes, which nothing waits on.
                xt = xp.tile([128, KC, T], F16, tag="xt", name="xt")
                nc.scalar.dma_start(
                    xt[:], xg[:, KC * t0:KC * (t0 + T)].rearrange(
                        "p (k t) -> p k t", k=KC))
                return xt

            DW_PRE = 4  # == dp bufs
            t0 = 0
            xt = _xt_dma(0, tiles[0])
            for i, T in enumerate(tiles):
                chunks = _chunks(T)
                wt = wpl.tile([128, T], F32, tag="wt")
                nc.scalar.dma_start(wt[:], wrep[:, t0:t0 + T])
                # one h tile per <=512-token chunk: keeps every tile under
                # 64KB/partition (a single [128, MC, 1536] fp16 tile faults
                # the exec unit)
                h_tiles = [hp.tile([128, MC, cl], F16, tag=f"h{ci}",
                                   name=f"h{ci}")
                           for ci, cl in enumerate(chunks)]
                for m in range(MC):
                    gw_t = gp.tile([128, KC, 128], F16, tag="gw")
                    nc.sync.dma_start(gw_t[:], gw[m])
                    uw_t = up.tile([128, KC, 128], F16, tag="uw")
                    nc.sync.dma_start(uw_t[:], uw[m])
                    c0 = 0
                    for ci, cl in enumerate(chunks):
                        gps = ps.tile([128, cl], F32, tag="gps", bufs=2)
                        for k in range(KC):
                            nc.tensor.matmul(
                                gps[:], gw_t[:, k, :], xt[:, k, c0:c0 + cl],
                                start=(k == 0), stop=(k == KC - 1))
                        ups = ps.tile([128, cl], F32, tag="ups", bufs=2)
                        for k in range(KC):
                            nc.tensor.matmul(
                                ups[:], uw_t[:, k, :], xt[:, k, c0:c0 + cl],
                                start=(k == 0), stop=(k == KC - 1))
                        sg = sgp.tile([128, cl], F16, tag="sg")
                        nc.scalar.activation(sg[:], gps[:], AF.Silu)
                        nc.vector.tensor_mul(h_tiles[ci][:, m, :],
                                             sg[:], ups[:])
                        c0 += cl
                # issue the first few dw DMAs, then the NEXT tile's x DMA,
                # before entering the d-loop: the x DMA instruction blocks
                # the sync ring until this tile's gate/up frees the xt
                # buffer, and the remaining dw issues resume right after --
                # so the next x streams during this tile's down phase
                # instead of stalling its start (~4us/tile otherwise)
                dw_pre = []
                for d in range(min(DW_PRE, DC)):
                    dw_t = dp.tile([128, MC, 128], F16, tag="dw")
                    nc.sync.dma_start(dw_t[:], dw[d])
                    dw_pre.append(dw_t)
                if i + 1 < len(tiles):
                    xt = _xt_dma(t0 + T, tiles[i + 1])
                for d in range(DC):
                    if d < len(dw_pre):
                        dw_t = dw_pre[d]
                    else:
                        dw_t = dp.tile([128, MC, 128], F16, tag="dw")
                        nc.sync.dma_start(dw_t[:], dw[d])
                    c0 = 0
                    for ci, cl in enumerate(chunks):
                        ops = ps.tile([128, cl], F32, tag="ops")
                        for m in range(MC):
                            nc.tensor.matmul(
                                ops[:], dw_t[:, m, :],
                                h_tiles[ci][:, m, :],
                                start=(m == 0), stop=(m == MC - 1))
                        ot = opl.tile([128, cl], F32, tag="ot")
                        nc.vector.tensor_mul(ot[:], ops[:], wt[:, c0:c0 + cl])
                        nc.scalar.dma_start(
                            outT[d * 128:(d + 1) * 128, t0 + c0:t0 + c0 + cl],
                            ot[:])
                        c0 += cl
                t0 += T
    nc.compile()
    return nc


def _pblock(a):
    """[R, C] with R = r*128 -> [128, r, C] so each SBUF partition reads
    contiguous data."""
    r = a.shape[0] // 128
    return np.ascontiguousarray(
        a.reshape(r, 128, a.shape[1]).transpose(1, 0, 2))


def kernel(x, router_w, gate_w, up_w, down_w):
    x = np.asarray(x, np.float32)
    router_w = np.asarray(router_w, np.float32)
    gate_w = np.asarray(gate_w, np.float32)
    up_w = np.asarray(up_w, np.float32)
    down_w = np.asarray(down_w, np.float32)

    import ml_dtypes
    BF = ml_dtypes.bfloat16

    x_flat = np.ascontiguousarray(x.reshape(N, D))
    rwT = np.ascontiguousarray(router_w.T)  # [D, E]
    rw_16 = rwT.astype(np.float16)

    # ---- launch 1: router logits (data-parallel over tokens) ----
    if "router" not in _cache:
        _cache["router"] = _build_router()
    nc_r = _cache["router"]
    xT_hi = x_flat.T.astype(np.float16)

    def _tile_major(a):  # [D, TOK_SHARD] -> [NT, 128, KC, 256]
        return np.ascontiguousarray(
            a.reshape(KC, 128, TOK_SHARD // 256, 256).transpose(2, 1, 0, 3))

    in_maps = [
        {
            "xs_hi": _tile_major(
                xT_hi[:, c * TOK_SHARD:(c + 1) * TOK_SHARD]),
            "rw": rw_16,
        }
        for c in range(N_CORES)
    ]
    res_r = _run(nc_r, in_maps, "launch1")
    logits = np.concatenate(
        [res_r[c]["logit"] for c in range(N_CORES)], 0).astype(np.float64)

    # Host dispatch math (softmax / top-2 / combine weights).  Device
    # logits carry ~1e-4 rms fp16 noise; the top-2 SET only flips when
    # the true top2/top3 logit gap is comparable, so recompute exactly
    # (in fp64, from the fp32 inputs) every token whose observed gap is
    # < 1.4e-2 (>100 sigma of the gap noise).  Deterministic, and host
    # cost is a few MFLOP (~800 tokens).
    ls = np.sort(logits, axis=1)
    risk = np.flatnonzero(ls[:, -2] - ls[:, -3] < 1.4e-2)
    if risk.size:
        logits[risk] = x_flat[risk].astype(np.float64) @ rwT.astype(np.float64)
    lmax = logits.max(axis=1, keepdims=True)
    p = np.exp(logits - lmax)
    p /= p.sum(axis=1, keepdims=True)
    top2 = np.argsort(-p, axis=1)[:, :2]
    rows = np.arange(N)
    p1 = p[rows, top2[:, 0]]
    p2 = p[rows, top2[:, 1]]
    combine = np.zeros((N, E), np.float32)
    combine[rows, top2[:, 0]] = (p1 / (p1 + p2)).astype(np.float32)
    combine[rows, top2[:, 1]] = (p2 / (p1 + p2)).astype(np.float32)

    # ---- host dispatch: token lists per expert, padded to capacity ----
    idx = [np.flatnonzero(combine[:, e] > 0.0) for e in range(E)]
    max_cnt = max(len(i) for i in idx)
    cap = max(512, max_cnt)
    tiles = _tile_sizes(cap)

    gw16 = gate_w.astype(np.float16)
    uw16 = up_w.astype(np.float16)
    dw16 = down_w.astype(np.float16)
    x16 = x_flat.astype(np.float16)
    in_maps = []
    for e in range(E):
        cnt = len(idx[e])
        xg = np.zeros((D, cap), np.float16)
        xg[:, :cnt] = x16[idx[e]].T
        # pack tile-major so each tile's x is one contiguous block per
        # partition (see _build_ffn docstring)
        xgp = _pblock(xg)  # [128, KC, cap]
        blocks, t0 = [], 0
        for T in tiles:
            blocks.append(np.ascontiguousarray(
                xgp[:, :, t0:t0 + T]).reshape(128, KC * T))
            t0 += T
        xg_flat = np.ascontiguousarray(np.concatenate(blocks, axis=1))
        wvec = np.zeros(cap, np.float32)
        wvec[:cnt] = combine[idx[e], e]
        wrep = np.ascontiguousarray(np.broadcast_to(wvec, (128, cap)))
        # gw[m, p, k, c] = gate_w[e][m*128+c, k*128+p]
        gwb = np.ascontiguousarray(
            gw16[e].reshape(MC, 128, KC, 128).transpose(0, 3, 2, 1))
        uwb = np.ascontiguousarray(
            uw16[e].reshape(MC, 128, KC, 128).transpose(0, 3, 2, 1))
        # dw[d, p, m, c] = down_w[e][d*128+c, m*128+p]
        dwb = np.ascontiguousarray(
            dw16[e].reshape(DC, 128, MC, 128).transpose(0, 3, 2, 1))
        in_maps.append({"xg": xg_flat, "gw": gwb, "uw": uwb, "dw": dwb,
                        "wrep": wrep})

    key = ("ffn", cap)
    if key not in _cache:
        _cache[key] = _build_ffn(cap, tiles)
    nc_f = _cache[key]
    res_f = _run(nc_f, in_maps, "launch2")

    # ---- host scatter-add ("all-to-all" return) ----
    out = np.zeros((N, D), np.float32)
    for e in range(E):
        cnt = len(idx[e])
        if cnt:
            out[idx[e]] += res_f[e]["outT"][:, :cnt].T
    return out.reshape(B, L, D)

